# revision 16
# baseline (speedup 1.0000x reference)
"""Trainium2 Bass kernel for nn_Encoder_5686536700540.

6-layer post-LN transformer encoder, B=4, S=1024, D=768, H=12, HD=64,
FFN hidden = 768, faithful "reshape-without-permute" bug after attention.

Sharding: DP over batch (4 pairs of cores) x TP2 over heads within a
pair. Core c handles batch b = c//2 and head half tp = c%2 (heads
tp*6..tp*6+5). The faithful-bug flat view [H,S,HD] -> [S, HID] means
heads 0-5 produce exactly view-rows 0-511 and heads 6-11 rows 512-1023,
so each core computes o@Wo for its own 512 rows; the per-layer exchange
is ONE bf16 AllGather of the (o@Wo + bo) delta per pair. Both cores
then apply the full-token residual + LNs + FFN redundantly, which keeps
the SPMD program identical on all cores (only input data differs).

On-chip layout is d-major ("transposed"): hT[d, t] as [128, 6, 1024].
All GEMMs compute Y.T = W.T @ X.T with W chunks [128,128] stationary.
Matmul operands are bf16 (weights pre-cast on the host; activations
cast on the producing write); the residual stream, PSUM accumulation,
LN statistics and softmax normalization stay fp32. LN reductions over
d (the partition dim) run on the TensorEngine via a ones-column
matmul; per-token stats are broadcast across partitions with GpSimd
partition_broadcast. Softmax: masked keys are handled by zeroing V
rows and using the 0/1 mask as the denominator reduction column
(reproduces exp(-1e20*SCALE)->0 exactly); exp/gelu/sqrt run on ACT.

fast_affine mode (host-verified: all projection biases zero, all LN
gammas one / betas zero -- true for this problem's setup_inputs): the
attn-LN (for l>0) and ffn-LN are skipped because their input is the
previous LN's output, which is already mean-0/std-1 per token: with
gamma=1, beta=0 the extra LN is the identity up to O(eps)=1e-5.
"""

import numpy as np
import ml_dtypes

import concourse.bass as bass
import concourse.mybir as mybir
import concourse.tile as tile
from concourse import bacc
from concourse.bass_utils import run_bass_kernel_spmd

B, S, D, H, HD, DEPTH, V = 4, 1024, 768, 12, 64, 6, 32000
HID = H * HD
EPS = 1e-5
SCALE = HD ** -0.5

HLOC = H // 2          # heads per core
DT = D // 128          # 6 d-tiles
CD = HLOC * HD         # 384 per-core qkv cols
CT = CD // 128         # 3 col tiles
SH = S // 2            # 512 view-rows per core
NT = S // 128          # 8 token tiles
F32 = mybir.dt.float32
BF16 = mybir.dt.bfloat16

AF = mybir.ActivationFunctionType
OP = mybir.AluOpType


def _bcast_ap(ap, parts):
    """[1, N] AP -> [parts, N] partition-broadcast AP (step 0)."""
    return bass.AP(tensor=ap.tensor, offset=ap.offset,
                   ap=[[0, parts]] + [list(x) for x in ap.ap[1:]])


def build(reps=1, use_cc=True, fast=True, skip=()):
    """Build the SPMD program (identical on all 8 cores; data differs)."""
    nc = bacc.Bacc(num_devices=8)

    h0T = nc.dram_tensor("h0T", [D, S], F32, kind="ExternalInput")
    m01 = nc.dram_tensor("m01", [128, NT], F32, kind="ExternalInput")
    wq = nc.dram_tensor("wq", [DEPTH, D, CD], BF16, kind="ExternalInput")
    wk = nc.dram_tensor("wk", [DEPTH, D, CD], BF16, kind="ExternalInput")
    wv = nc.dram_tensor("wv", [DEPTH, D, CD], BF16, kind="ExternalInput")
    wo = nc.dram_tensor("wo", [DEPTH, D, D], BF16, kind="ExternalInput")
    w1 = nc.dram_tensor("w1", [DEPTH, D, D], BF16, kind="ExternalInput")
    w2 = nc.dram_tensor("w2", [DEPTH, D, D], BF16, kind="ExternalInput")
    bq = nc.dram_tensor("bq", [DEPTH, 128, CT], F32, kind="ExternalInput")
    bk = nc.dram_tensor("bk", [DEPTH, 128, CT], F32, kind="ExternalInput")
    bv = nc.dram_tensor("bv", [DEPTH, 1, CD], F32, kind="ExternalInput")
    bo = nc.dram_tensor("bo", [DEPTH, 128, DT], F32, kind="ExternalInput")
    b1 = nc.dram_tensor("b1", [DEPTH, 128, DT], F32, kind="ExternalInput")
    b2 = nc.dram_tensor("b2", [DEPTH, 128, DT], F32, kind="ExternalInput")
    ag_g = nc.dram_tensor("ag_g", [DEPTH, 128, DT], F32, kind="ExternalInput")
    ag_b = nc.dram_tensor("ag_b", [DEPTH, 128, DT], F32, kind="ExternalInput")
    fg_g = nc.dram_tensor("fg_g", [DEPTH, 128, DT], F32, kind="ExternalInput")
    fg_b = nc.dram_tensor("fg_b", [DEPTH, 128, DT], F32, kind="ExternalInput")
    en_g = nc.dram_tensor("en_g", [128, DT], F32, kind="ExternalInput")
    en_b = nc.dram_tensor("en_b", [128, DT], F32, kind="ExternalInput")

    outT = nc.dram_tensor("outT", [D, S], F32, kind="ExternalOutput")

    o_scr = nc.dram_tensor("o_scr", [HLOC, S, HD], BF16)
    cc_in = [nc.dram_tensor(f"cc_in_{i}", [D, SH], BF16)
             for i in range(reps * DEPTH)]
    cc_out = [nc.dram_tensor(f"cc_out_{i}", [2 * D, SH], BF16)
              for i in range(reps * DEPTH)]
    groups = [[0, 1], [2, 3], [4, 5], [6, 7]]

    from concourse.masks import make_identity
    import contextlib

    with tile.TileContext(nc) as tc:
        with contextlib.ExitStack() as ctx:
            sb = ctx.enter_context(tc.tile_pool(name="sb", bufs=1))
            wp = ctx.enter_context(tc.tile_pool(name="wp", bufs=3))
            sm = ctx.enter_context(tc.tile_pool(name="sm", bufs=2))
            st_pool = ctx.enter_context(tc.tile_pool(name="st", bufs=1))
            hot = ctx.enter_context(tc.tile_pool(name="hot", bufs=3))
            ps = ctx.enter_context(tc.tile_pool(name="ps", bufs=3,
                                                space="PSUM"))
            scp = ctx.enter_context(tc.tile_pool(name="scp", bufs=2,
                                                 space="PSUM"))
            ps2 = ctx.enter_context(tc.tile_pool(name="ps2", bufs=1,
                                                 space="PSUM"))

            # ---- constants ----
            ident = sb.tile([128, 128], BF16, tag="ident")
            make_identity(nc, ident)
            ones_col = sb.tile([128, 1], F32, tag="ones_col")
            nc.vector.memset(ones_col, 1.0)
            ones_bf = sb.tile([128, 1], BF16, tag="ones_bf")
            nc.vector.memset(ones_bf, 1.0)
            eps_t = sb.tile([128, 1], F32, tag="eps")
            nc.vector.memset(eps_t, EPS)
            m01_sb = sb.tile([128, NT], F32, tag="m01")
            nc.sync.dma_start(out=m01_sb, in_=m01[:])
            m01b = sb.tile([128, NT], BF16, tag="m01b")
            nc.vector.tensor_copy(out=m01b, in_=m01_sb)
            eng_sb = sb.tile([128, DT], F32, tag="eng")
            nc.sync.dma_start(out=eng_sb, in_=en_g[:])
            enb_sb = sb.tile([128, DT], F32, tag="enb")
            nc.sync.dma_start(out=enb_sb, in_=en_b[:])

            # persistent activations (bytes/partition in parens)
            hT = sb.tile([128, DT, S], F32, tag="hT")        # 24K
            sb_dT_bf = sb.tile([128, DT, S], BF16, tag="dT")  # 12K deltas
            dTb = sb.tile([128, DT, SH], BF16, tag="dTb")    # 6K
            nTb = sb.tile([128, DT, S], BF16, tag="nTb")     # 12K (LN out)
            qTb = sb.tile([128, CT, S], BF16, tag="qTb")     # 6K
            kTb = sb.tile([128, CT, S], BF16, tag="kTb")     # 6K
            vtkb = sb.tile([128, NT, CD], BF16, tag="vtkb")  # 6K
            otok = sb.tile([128, NT, CD], BF16, tag="otok")  # 6K
            ovT = sb.tile([128, DT, SH], BF16, tag="ovT")    # 6K
            gT = sb.tile([128, DT, SH], BF16, tag="gT")      # 6K
            hb = sb.tile([128, DT, SH], BF16, tag="hb")      # 6K
            rb_sb = sb.tile([128, SH], F32, tag="rb_sb")     # 2K
            mb_sb = sb.tile([128, SH], F32, tag="mb_sb")     # 2K

            if skip:
                # ablation builds read some tiles that are never written;
                # give every big tile a defined initial value
                for t in (hT, sb_dT_bf, dTb, nTb, qTb, kTb, vtkb, otok, ovT,
                          gT, hb, rb_sb, mb_sb):
                    nc.vector.memset(t, 0.25)

            def layer_norm(src, dst, g_ap, b_ap):
                """dst = LN(src) over d (full tokens). src [128,DT,S] f32;
                dst may be f32 (hT, in place) or bf16 (nTb).

                When g_ap is None (fast mode) the gamma/beta pass is
                skipped. Uses hb + gT as scratch; must not be called
                while gT holds live data.
                """
                if "ln" in skip:
                    return
                for hf in range(2):
                    sl = slice(hf * SH, (hf + 1) * SH)
                    # bf16 copy of src for the PE reduction (GpSimd)
                    for kt in range(DT):
                        nc.gpsimd.tensor_copy(out=hb[:, kt, :],
                                              in_=src[:, kt, sl])
                    st = ps2.tile([33, SH], F32, tag="ln_st")
                    for kt in range(DT):
                        nc.tensor.matmul(st[0:1, :], lhsT=ones_bf,
                                         rhs=hb[:, kt, :],
                                         start=(kt == 0), stop=(kt == DT - 1),
                                         tile_position=(0, 0),
                                         skip_group_check=True)
                    for kt in range(DT):
                        nc.vector.tensor_tensor(out=gT[:, kt, :],
                                                in0=hb[:, kt, :],
                                                in1=hb[:, kt, :],
                                                op=OP.mult)
                    for kt in range(DT):
                        nc.tensor.matmul(st[32:33, :], lhsT=ones_bf,
                                         rhs=gT[:, kt, :],
                                         start=(kt == 0), stop=(kt == DT - 1),
                                         tile_position=(0, 32),
                                         skip_group_check=True)
                    # per-token stats on one partition (f32)
                    mrow = st_pool.tile([1, SH], F32, tag="ln_m")
                    nc.vector.tensor_scalar_mul(out=mrow, in0=st[0:1, :],
                                                scalar1=1.0 / D)
                    vrow = st_pool.tile([1, SH], F32, tag="ln_v")
                    nc.vector.tensor_scalar_mul(out=vrow, in0=st[32:33, :],
                                                scalar1=1.0 / D)
                    msq = st_pool.tile([1, SH], F32, tag="ln_msq")
                    nc.vector.tensor_tensor(out=msq, in0=mrow, in1=mrow,
                                            op=OP.mult)
                    nc.vector.tensor_tensor(out=vrow, in0=vrow, in1=msq,
                                            op=OP.subtract)
                    nc.scalar.activation(out=vrow, in_=vrow, func=AF.Sqrt,
                                         bias=eps_t[0:1], scale=1.0)
                    nc.vector.reciprocal(out=vrow, in_=vrow)      # r
                    nc.vector.tensor_tensor(out=mrow, in0=mrow, in1=vrow,
                                            op=OP.mult)           # m*r
                    nc.gpsimd.partition_broadcast(rb_sb, vrow)
                    nc.gpsimd.partition_broadcast(mb_sb, mrow)
                    # dst = (x*r - m*r) [* g + b]
                    for kt in range(DT):
                        nc.vector.tensor_tensor(out=dst[:, kt, sl],
                                                in0=src[:, kt, sl],
                                                in1=rb_sb, op=OP.mult)
                        nc.vector.tensor_tensor(out=dst[:, kt, sl],
                                                in0=dst[:, kt, sl],
                                                in1=mb_sb, op=OP.subtract)
                        if g_ap is not None:
                            nc.vector.tensor_scalar(
                                out=dst[:, kt, sl], in0=dst[:, kt, sl],
                                scalar1=g_ap[:, kt:kt + 1],
                                scalar2=b_ap[:, kt:kt + 1],
                                op0=OP.mult, op1=OP.add)

            def cast_to_nTb():
                for kt in range(DT):
                    nc.gpsimd.tensor_copy(out=nTb[:, kt, :],
                                          in_=hT[:, kt, :])

            for rep in range(reps):
                nc.sync.dma_start(
                    out=hT, in_=h0T[:].rearrange("(kt p) t -> p kt t", p=128))
                for l in range(DEPTH):
                    # ---- per-layer small tensors ----
                    if not fast:
                        agg = sm.tile([128, DT], F32, tag="agg")
                        nc.sync.dma_start(out=agg, in_=ag_g[l])
                        agb = sm.tile([128, DT], F32, tag="agb")
                        nc.sync.dma_start(out=agb, in_=ag_b[l])
                        fgg = sm.tile([128, DT], F32, tag="fgg")
                        nc.sync.dma_start(out=fgg, in_=fg_g[l])
                        fgb = sm.tile([128, DT], F32, tag="fgb")
                        nc.sync.dma_start(out=fgb, in_=fg_b[l])
                        bq_t = sm.tile([128, CT], F32, tag="bq")
                        nc.sync.dma_start(out=bq_t, in_=bq[l])
                        bk_t = sm.tile([128, CT], F32, tag="bk")
                        nc.sync.dma_start(out=bk_t, in_=bk[l])
                        bvb = sm.tile([128, CD], F32, tag="bvb")
                        nc.sync.dma_start(out=bvb, in_=_bcast_ap(bv[l], 128))
                        bo_t = sm.tile([128, DT], F32, tag="bo")
                        nc.sync.dma_start(out=bo_t, in_=bo[l])
                        b1_t = sm.tile([128, DT], F32, tag="b1")
                        nc.sync.dma_start(out=b1_t, in_=b1[l])
                        b2_t = sm.tile([128, DT], F32, tag="b2")
                        nc.sync.dma_start(out=b2_t, in_=b2[l])

                    # ---- LN_attn -> nTb (skipped for l>0 in fast mode:
                    # input is already the previous enc-LN output) ----
                    if fast:
                        if l == 0:
                            layer_norm(hT, nTb, None, None)
                        else:
                            cast_to_nTb()
                    else:
                        layer_norm(hT, nTb, agg, agb)

                    # ---- Q, K projections (col-major YT form) ----
                    for (w_dram, b_dram, dstT) in (
                            () if "qkv" in skip else ((wq, bq, qTb),
                                                      (wk, bk, kTb))):
                        w_sb = wp.tile([128, DT, D], BF16, tag="w")
                        nc.sync.dma_start(
                            out=w_sb[:, :, 0:CD],
                            in_=w_dram[l].rearrange("(kt p) c -> p kt c",
                                                    p=128))
                        for wc in range(CT):
                            for hf in range(2):
                                sl = slice(hf * SH, (hf + 1) * SH)
                                acc = ps.tile([128, SH], F32, tag="ps")
                                for kt in range(DT):
                                    nc.tensor.matmul(
                                        acc,
                                        lhsT=w_sb[:, kt,
                                                  wc * 128:(wc + 1) * 128],
                                        rhs=nTb[:, kt, sl],
                                        start=(kt == 0),
                                        stop=(kt == DT - 1))
                                if fast:
                                    nc.vector.tensor_copy(
                                        out=dstT[:, wc, sl], in_=acc)
                                else:
                                    b_t = sm.tile([128, CT], F32,
                                                  tag="bq" if dstT is qTb
                                                  else "bk")
                                    nc.vector.tensor_scalar(
                                        out=dstT[:, wc, sl], in0=acc,
                                        scalar1=b_t[:, wc:wc + 1],
                                        scalar2=None, op0=OP.add)

                    # ---- V projection (token-major) + bias + mask ----
                    w_sb = wp.tile([128, DT, D], BF16, tag="w")
                    nc.sync.dma_start(
                        out=w_sb[:, :, 0:CD],
                        in_=wv[l].rearrange("(kt p) c -> p kt c", p=128))
                    for tt in (() if "qkv" in skip else range(NT)):
                        acc = ps.tile([128, SH], F32, tag="ps")
                        for kt in range(DT):
                            nc.tensor.matmul(
                                acc[:, 0:CD],
                                lhsT=nTb[:, kt, tt * 128:(tt + 1) * 128],
                                rhs=w_sb[:, kt, 0:CD],
                                start=(kt == 0), stop=(kt == DT - 1))
                        if not fast:
                            bvb = sm.tile([128, CD], F32, tag="bvb")
                            nc.vector.tensor_tensor(
                                out=acc[:, 0:CD], in0=acc[:, 0:CD],
                                in1=bvb, op=OP.add)
                        nc.vector.tensor_scalar_mul(
                            out=vtkb[:, tt, :], in0=acc[:, 0:CD],
                            scalar1=m01_sb[:, tt:tt + 1])

                    # ---- attention per head ----
                    for h in (() if "attn" in skip else range(HLOC)):
                        hr = (h % 2) * 64
                        h3 = h // 2
                        for qh in range(2):
                            qsl = slice(qh * SH, (qh + 1) * SH)
                            o_ps = ps.tile([65, SH], F32, tag="ps")
                            for kt2 in range(NT // 2):
                                s_ps = scp.tile([128, 2, SH], F32, tag="sc")
                                for j in range(2):
                                    kt = 2 * kt2 + j
                                    nc.tensor.matmul(
                                        s_ps[:, j, :],
                                        lhsT=kTb[hr:hr + 64, h3,
                                                 kt * 128:(kt + 1) * 128],
                                        rhs=qTb[hr:hr + 64, h3, qsl],
                                        start=True, stop=True)
                                eT = hot.tile([128, 2, SH], BF16, tag="eT")
                                nc.scalar.activation(out=eT, in_=s_ps,
                                                     func=AF.Exp,
                                                     scale=SCALE)
                                for j in range(2):
                                    kt = 2 * kt2 + j
                                    nc.tensor.matmul(
                                        o_ps[0:64, :],
                                        lhsT=vtkb[:, kt,
                                                  h * 64:(h + 1) * 64],
                                        rhs=eT[:, j, :],
                                        start=(kt == 0),
                                        stop=(kt == NT - 1),
                                        tile_position=(0, 0),
                                        skip_group_check=True)
                                    nc.tensor.matmul(
                                        o_ps[64:65, :],
                                        lhsT=m01b[:, kt:kt + 1],
                                        rhs=eT[:, j, :],
                                        start=(kt == 0),
                                        stop=(kt == NT - 1),
                                        tile_position=(0, 64),
                                        skip_group_check=True)
                            rs = st_pool.tile([1, SH], F32, tag="rs")
                            nc.vector.reciprocal(out=rs, in_=o_ps[64:65, :])
                            ou = hot.tile([64, SH], BF16, tag="on")
                            nc.vector.tensor_copy(out=ou, in_=o_ps[0:64, :])
                            for qc in range(SH // 128):
                                # per-token 1/sum as a [128,1] column
                                rT_ps = ps.tile([128, 1], F32, tag="ps")
                                nc.tensor.matmul(
                                    rT_ps,
                                    lhsT=rs[:, qc * 128:(qc + 1) * 128],
                                    rhs=ones_col[0:1, 0:1],
                                    start=True, stop=True)
                                rT_sb = st_pool.tile([128, 1], F32,
                                                     tag="rT")
                                nc.vector.tensor_copy(out=rT_sb, in_=rT_ps)
                                tp_ps = ps.tile([128, 64], BF16, tag="ps")
                                nc.tensor.transpose(
                                    tp_ps, ou[:, qc * 128:(qc + 1) * 128],
                                    ident[0:64, 0:64])
                                nc.vector.tensor_scalar_mul(
                                    out=otok[:, qh * (SH // 128) + qc,
                                             h * 64:(h + 1) * 64],
                                    in0=tp_ps, scalar1=rT_sb)
                        # spill head output to DRAM in flat [h, s, hd] order
                        nc.sync.dma_start(
                            out=o_scr[h].rearrange("(tt p) d -> p tt d",
                                                   p=128),
                            in_=otok[:, :, h * 64:(h + 1) * 64])

                    # ---- scrambled [SH, D] view, transpose to col-major ----
                    oview = o_scr[:].rearrange("h s d -> (h s d)").rearrange(
                        "(r c) -> r c", c=D)
                    for rt in (() if "obounce" in skip else range(SH // 128)):
                        ov_tok = hot.tile([128, D], BF16, tag="ov_tok")
                        nc.sync.dma_start(
                            out=ov_tok, in_=oview[rt * 128:(rt + 1) * 128, :])
                        for ct in range(DT):
                            tp_ps = ps.tile([128, 128], BF16, tag="ps")
                            nc.tensor.transpose(
                                tp_ps, ov_tok[:, ct * 128:(ct + 1) * 128],
                                ident)
                            nc.vector.tensor_copy(
                                out=ovT[:, ct, rt * 128:(rt + 1) * 128],
                                in_=tp_ps)

                    # ---- Wo GEMM -> delta for my 512 rows (+bo) -> dTb ----
                    w_sb = wp.tile([128, DT, D], BF16, tag="w")
                    nc.sync.dma_start(
                        out=w_sb,
                        in_=wo[l].rearrange("(ct p) w -> p ct w", p=128))
                    for wc in (() if "wo" in skip else range(DT)):
                        acc = ps.tile([128, SH], F32, tag="ps")
                        for ct in range(DT):
                            nc.tensor.matmul(
                                acc,
                                lhsT=w_sb[:, ct, wc * 128:(wc + 1) * 128],
                                rhs=ovT[:, ct, :],
                                start=(ct == 0), stop=(ct == DT - 1))
                        if fast:
                            nc.vector.tensor_copy(out=dTb[:, wc, :], in_=acc)
                        else:
                            bo_t = sm.tile([128, DT], F32, tag="bo")
                            nc.vector.tensor_scalar(
                                out=dTb[:, wc, :], in0=acc,
                                scalar1=bo_t[:, wc:wc + 1], scalar2=None,
                                op0=OP.add)

                    # ---- AllGather deltas within the pair (bf16) ----
                    if "ag" not in skip:
                        ci = cc_in[rep * DEPTH + l]
                        co = cc_out[rep * DEPTH + l]
                        nc.sync.dma_start(
                            out=ci[:].rearrange("(kt p) t -> p kt t", p=128),
                            in_=dTb)
                        if use_cc:
                            nc.gpsimd.collective_compute(
                                "AllGather", OP.bypass, ins=[ci[:]],
                                outs=[co[:]], replica_groups=groups)
                        else:
                            # timeline-sim variant: fake exchange locally
                            nc.sync.dma_start(out=co[0:D, :], in_=ci[:])
                            nc.sync.dma_start(out=co[D:2 * D, :], in_=ci[:])
                        cov = co[:].rearrange("(blk kt p) t -> blk p kt t",
                                              blk=2, p=128)
                        dTf = sb_dT_bf
                        for blk in range(2):
                            nc.sync.dma_start(
                                out=dTf[:, :, blk * SH:(blk + 1) * SH],
                                in_=cov[blk])
                        # full residual: h += delta (bf16 in1)
                        for kt in range(DT):
                            nc.vector.tensor_tensor(out=hT[:, kt, :],
                                                    in0=hT[:, kt, :],
                                                    in1=dTf[:, kt, :],
                                                    op=OP.add)

                    # ---- post-attn enc LN (full, in place) ----
                    layer_norm(hT, hT, None if fast else eng_sb,
                               None if fast else enb_sb)

                    # ---- FFN (full tokens, redundant across the pair) ----
                    if fast:
                        cast_to_nTb()   # ffn-LN == identity here
                    else:
                        layer_norm(hT, nTb, fgg, fgb)
                    w1_sb = wp.tile([128, DT, D], BF16, tag="w")
                    nc.sync.dma_start(
                        out=w1_sb,
                        in_=w1[l].rearrange("(kt p) c -> p kt c", p=128))
                    w2_sb = wp.tile([128, DT, D], BF16, tag="w")
                    nc.sync.dma_start(
                        out=w2_sb,
                        in_=w2[l].rearrange("(kt p) c -> p kt c", p=128))
                    for hf in (() if "ffn" in skip else range(2)):
                        sl = slice(hf * SH, (hf + 1) * SH)
                        for hc in range(DT):
                            acc = ps.tile([128, SH], F32, tag="ps")
                            for kt in range(DT):
                                nc.tensor.matmul(
                                    acc,
                                    lhsT=w1_sb[:, kt,
                                               hc * 128:(hc + 1) * 128],
                                    rhs=nTb[:, kt, sl],
                                    start=(kt == 0), stop=(kt == DT - 1))
                            if fast:
                                nc.scalar.activation(out=gT[:, hc, :],
                                                     in_=acc, func=AF.Gelu,
                                                     scale=1.0)
                            else:
                                b1_t = sm.tile([128, DT], F32, tag="b1")
                                nc.scalar.activation(out=gT[:, hc, :],
                                                     in_=acc, func=AF.Gelu,
                                                     bias=b1_t[:, hc:hc + 1],
                                                     scale=1.0)
                        for wc in range(DT):
                            acc = ps.tile([128, SH], F32, tag="ps")
                            for kt in range(DT):
                                nc.tensor.matmul(
                                    acc,
                                    lhsT=w2_sb[:, kt,
                                               wc * 128:(wc + 1) * 128],
                                    rhs=gT[:, kt, :],
                                    start=(kt == 0), stop=(kt == DT - 1))
                            if not fast:
                                b2_t = sm.tile([128, DT], F32, tag="b2")
                                nc.vector.tensor_scalar(
                                    out=acc, in0=acc,
                                    scalar1=b2_t[:, wc:wc + 1],
                                    scalar2=None, op0=OP.add)
                            nc.vector.tensor_tensor(
                                out=hT[:, wc, sl], in0=hT[:, wc, sl],
                                in1=acc, op=OP.add)

                    # ---- post-FFN enc LN (full, in place) ----
                    layer_norm(hT, hT, None if fast else eng_sb,
                               None if fast else enb_sb)

            nc.sync.dma_start(
                out=outT[:].rearrange("(kt p) t -> p kt t", p=128),
                in_=hT)

    nc.finalize()
    return nc


def _per_core_inputs(inputs, core):
    """Host-side sharding: build this core's input map."""
    b, tp = core // 2, core % 2
    csl = slice(tp * CD, (tp + 1) * CD)

    x = np.asarray(inputs["x"])
    emb = np.asarray(inputs["emb"], dtype=np.float32)
    pos = np.asarray(inputs["pos_embed"], dtype=np.float32)[0]
    h0 = emb[x[b]] + pos                                # [S, D]
    mask = np.asarray(inputs["mask"])[b, 0, 0]          # [S]

    def pp(v, dt_):  # [DEPTH, 768] -> [DEPTH, 128, 6]
        v = np.asarray(v, dtype=np.float32)
        return np.ascontiguousarray(
            v.reshape(DEPTH, dt_, 128).transpose(0, 2, 1))

    def bf(v):
        return np.ascontiguousarray(
            np.asarray(v, np.float32).astype(ml_dtypes.bfloat16))

    return {
        "h0T": np.ascontiguousarray(h0.T),
        "m01": np.ascontiguousarray(
            mask.reshape(NT, 128).T.astype(np.float32)),
        "wq": bf(np.asarray(inputs["Wq"], np.float32)[:, :, csl]),
        "wk": bf(np.asarray(inputs["Wk"], np.float32)[:, :, csl]),
        "wv": bf(np.asarray(inputs["Wv"], np.float32)[:, :, csl]),
        "wo": bf(inputs["Wo"]),
        "w1": bf(inputs["W1"]),
        "w2": bf(inputs["W2"]),
        "bq": np.ascontiguousarray(
            np.asarray(inputs["bq"], np.float32)[:, csl]
            .reshape(DEPTH, CT, 128).transpose(0, 2, 1)),
        "bk": np.ascontiguousarray(
            np.asarray(inputs["bk"], np.float32)[:, csl]
            .reshape(DEPTH, CT, 128).transpose(0, 2, 1)),
        "bv": np.ascontiguousarray(
            np.asarray(inputs["bv"], np.float32)[:, csl]
            .reshape(DEPTH, 1, CD)),
        "bo": pp(inputs["bo"], DT),
        "b1": pp(inputs["b1"], DT),
        "b2": pp(inputs["b2"], DT),
        "ag_g": pp(inputs["attn_g"], DT),
        "ag_b": pp(inputs["attn_b"], DT),
        "fg_g": pp(inputs["ff_g"], DT),
        "fg_b": pp(inputs["ff_b"], DT),
        "en_g": np.ascontiguousarray(
            np.asarray(inputs["enc_g"], np.float32).reshape(DT, 128).T),
        "en_b": np.ascontiguousarray(
            np.asarray(inputs["enc_b"], np.float32).reshape(DT, 128).T),
    }


def _is_fast_affine(inputs):
    z = lambda k: not np.any(np.asarray(inputs[k], np.float32))
    o = lambda k: np.all(np.asarray(inputs[k], np.float32) == 1.0)
    return (z("bq") and z("bk") and z("bv") and z("bo") and z("b1")
            and z("b2") and z("attn_b") and z("ff_b") and z("enc_b")
            and o("attn_g") and o("ff_g") and o("enc_g"))


_CACHE = {}


def _get_nc(reps=1, fast=True):
    key = ("nc", reps, fast)
    if key not in _CACHE:
        _CACHE[key] = build(reps=reps, fast=fast)
    return _CACHE[key]


def kernel(**inputs):
    fast = _is_fast_affine(inputs)
    nc = _get_nc(reps=1, fast=fast)
    in_maps = [_per_core_inputs(inputs, c) for c in range(8)]
    res = run_bass_kernel_spmd(nc, in_maps, list(range(8)))
    out = np.empty((B, S, D), dtype=np.float32)
    for b in range(B):
        out[b] = res.results[2 * b]["outT"].T
    return out


# revision 17
# speedup vs baseline: 1.1200x; 1.1200x over previous
"""Trainium2 Bass kernel for nn_Encoder_5686536700540.

6-layer post-LN transformer encoder, B=4, S=1024, D=768, H=12, HD=64,
FFN hidden = 768, faithful "reshape-without-permute" bug after attention.

Sharding: DP over batch (4 pairs of cores) x TP2 over heads within a
pair. Core c handles batch b = c//2 and head half tp = c%2 (heads
tp*6..tp*6+5). The faithful-bug flat view [H,S,HD] -> [S, HID] means
heads 0-5 produce exactly view-rows 0-511 and heads 6-11 rows 512-1023,
so each core computes o@Wo for its own 512 rows; the per-layer exchange
is ONE bf16 AllGather of the (o@Wo + bo) delta per pair. Both cores
then apply the full-token residual + LNs + FFN redundantly, which keeps
the SPMD program identical on all cores (only input data differs).

On-chip layout is d-major ("transposed"): hT[d, t] as [128, 6, 1024].
All GEMMs compute Y.T = W.T @ X.T with W chunks [128,128] stationary.
Matmul operands are bf16 (weights pre-cast on the host; activations
cast on the producing write); the residual stream, PSUM accumulation,
LN statistics and softmax normalization stay fp32. LN reductions over
d (the partition dim) run on the TensorEngine via a ones-column
matmul; per-token stats are broadcast across partitions with GpSimd
partition_broadcast. Softmax: masked keys are handled by zeroing V
rows and using the 0/1 mask as the denominator reduction column
(reproduces exp(-1e20*SCALE)->0 exactly); exp/gelu/sqrt run on ACT.

fast_affine mode (host-verified: all projection biases zero, all LN
gammas one / betas zero -- true for this problem's setup_inputs): the
attn-LN (for l>0) and ffn-LN are skipped because their input is the
previous LN's output, which is already mean-0/std-1 per token: with
gamma=1, beta=0 the extra LN is the identity up to O(eps)=1e-5.
"""

import numpy as np
import ml_dtypes

import concourse.bass as bass
import concourse.mybir as mybir
import concourse.tile as tile
from concourse import bacc
from concourse.bass_utils import run_bass_kernel_spmd

B, S, D, H, HD, DEPTH, V = 4, 1024, 768, 12, 64, 6, 32000
HID = H * HD
EPS = 1e-5
SCALE = HD ** -0.5

HLOC = H // 2          # heads per core
DT = D // 128          # 6 d-tiles
CD = HLOC * HD         # 384 per-core qkv cols
CT = CD // 128         # 3 col tiles
SH = S // 2            # 512 view-rows per core
NT = S // 128          # 8 token tiles
F32 = mybir.dt.float32
BF16 = mybir.dt.bfloat16

AF = mybir.ActivationFunctionType
OP = mybir.AluOpType


def _bcast_ap(ap, parts):
    """[1, N] AP -> [parts, N] partition-broadcast AP (step 0)."""
    return bass.AP(tensor=ap.tensor, offset=ap.offset,
                   ap=[[0, parts]] + [list(x) for x in ap.ap[1:]])


def build(reps=1, use_cc=True, fast=True, skip=()):
    """Build the SPMD program (identical on all 8 cores; data differs)."""
    nc = bacc.Bacc(num_devices=8)

    h0T = nc.dram_tensor("h0T", [D, S], F32, kind="ExternalInput")
    m01 = nc.dram_tensor("m01", [128, NT], F32, kind="ExternalInput")
    wq = nc.dram_tensor("wq", [DEPTH, D, CD], BF16, kind="ExternalInput")
    wk = nc.dram_tensor("wk", [DEPTH, D, CD], BF16, kind="ExternalInput")
    wv = nc.dram_tensor("wv", [DEPTH, D, CD], BF16, kind="ExternalInput")
    wo = nc.dram_tensor("wo", [DEPTH, D, D], BF16, kind="ExternalInput")
    w1 = nc.dram_tensor("w1", [DEPTH, D, D], BF16, kind="ExternalInput")
    w2 = nc.dram_tensor("w2", [DEPTH, D, D], BF16, kind="ExternalInput")
    bq = nc.dram_tensor("bq", [DEPTH, 128, CT], F32, kind="ExternalInput")
    bk = nc.dram_tensor("bk", [DEPTH, 128, CT], F32, kind="ExternalInput")
    bv = nc.dram_tensor("bv", [DEPTH, 1, CD], F32, kind="ExternalInput")
    bo = nc.dram_tensor("bo", [DEPTH, 128, DT], F32, kind="ExternalInput")
    b1 = nc.dram_tensor("b1", [DEPTH, 128, DT], F32, kind="ExternalInput")
    b2 = nc.dram_tensor("b2", [DEPTH, 128, DT], F32, kind="ExternalInput")
    ag_g = nc.dram_tensor("ag_g", [DEPTH, 128, DT], F32, kind="ExternalInput")
    ag_b = nc.dram_tensor("ag_b", [DEPTH, 128, DT], F32, kind="ExternalInput")
    fg_g = nc.dram_tensor("fg_g", [DEPTH, 128, DT], F32, kind="ExternalInput")
    fg_b = nc.dram_tensor("fg_b", [DEPTH, 128, DT], F32, kind="ExternalInput")
    en_g = nc.dram_tensor("en_g", [128, DT], F32, kind="ExternalInput")
    en_b = nc.dram_tensor("en_b", [128, DT], F32, kind="ExternalInput")

    outT = nc.dram_tensor("outT", [D, S], F32, kind="ExternalOutput")

    o_scr = nc.dram_tensor("o_scr", [HLOC, S, HD], BF16)
    cc_in = [nc.dram_tensor(f"cc_in_{i}", [D, SH], BF16)
             for i in range(reps * DEPTH)]
    cc_out = [nc.dram_tensor(f"cc_out_{i}", [2 * D, SH], BF16)
              for i in range(reps * DEPTH)]
    groups = [[0, 1], [2, 3], [4, 5], [6, 7]]

    from concourse.masks import make_identity
    import contextlib

    with tile.TileContext(nc) as tc:
        with contextlib.ExitStack() as ctx:
            sb = ctx.enter_context(tc.tile_pool(name="sb", bufs=1))
            wp = ctx.enter_context(tc.tile_pool(name="wp", bufs=4))
            sm = ctx.enter_context(tc.tile_pool(name="sm", bufs=2))
            st_pool = ctx.enter_context(tc.tile_pool(name="st", bufs=1))
            hot = ctx.enter_context(tc.tile_pool(name="hot", bufs=3))
            ps = ctx.enter_context(tc.tile_pool(name="ps", bufs=3,
                                                space="PSUM"))
            scp = ctx.enter_context(tc.tile_pool(name="scp", bufs=2,
                                                 space="PSUM"))
            ps2 = ctx.enter_context(tc.tile_pool(name="ps2", bufs=1,
                                                 space="PSUM"))

            # ---- constants ----
            ident = sb.tile([128, 128], BF16, tag="ident")
            make_identity(nc, ident)
            ones_col = sb.tile([128, 1], F32, tag="ones_col")
            nc.vector.memset(ones_col, 1.0)
            ones_bf = sb.tile([128, 1], BF16, tag="ones_bf")
            nc.vector.memset(ones_bf, 1.0)
            eps_t = sb.tile([128, 1], F32, tag="eps")
            nc.vector.memset(eps_t, EPS)
            m01_sb = sb.tile([128, NT], F32, tag="m01")
            nc.sync.dma_start(out=m01_sb, in_=m01[:])
            m01b = sb.tile([128, NT], BF16, tag="m01b")
            nc.vector.tensor_copy(out=m01b, in_=m01_sb)
            eng_sb = sb.tile([128, DT], F32, tag="eng")
            nc.sync.dma_start(out=eng_sb, in_=en_g[:])
            enb_sb = sb.tile([128, DT], F32, tag="enb")
            nc.sync.dma_start(out=enb_sb, in_=en_b[:])

            # persistent activations (bytes/partition in parens)
            hT = sb.tile([128, DT, S], F32, tag="hT")        # 24K
            sb_dT_bf = sb.tile([128, DT, S], BF16, tag="dT")  # 12K deltas
            dTb = sb.tile([128, DT, SH], BF16, tag="dTb")    # 6K
            nTb = sb.tile([128, DT, S], BF16, tag="nTb")     # 12K (LN out)
            qTb = sb.tile([128, CT, S], BF16, tag="qTb")     # 6K
            kTb = sb.tile([128, CT, S], BF16, tag="kTb")     # 6K
            vtkb = sb.tile([128, NT, CD], BF16, tag="vtkb")  # 6K
            otok = sb.tile([128, NT, CD], BF16, tag="otok")  # 6K
            ovT = sb.tile([128, DT, SH], BF16, tag="ovT")    # 6K
            gT = sb.tile([128, DT, SH], BF16, tag="gT")      # 6K
            hb = sb.tile([128, DT, SH], BF16, tag="hb")      # 6K
            rb_sb = sb.tile([128, SH], F32, tag="rb_sb")     # 2K
            mb_sb = sb.tile([128, SH], F32, tag="mb_sb")     # 2K

            if skip:
                # ablation builds read some tiles that are never written;
                # give every big tile a defined initial value
                for t in (hT, sb_dT_bf, dTb, nTb, qTb, kTb, vtkb, otok, ovT,
                          gT, hb, rb_sb, mb_sb):
                    nc.vector.memset(t, 0.25)

            def layer_norm(src, dst, g_ap, b_ap):
                """dst = LN(src) over d (full tokens). src [128,DT,S] f32;
                dst may be f32 (hT, in place) or bf16 (nTb).

                When g_ap is None (fast mode) the gamma/beta pass is
                skipped. Uses hb + gT as scratch; must not be called
                while gT holds live data.
                """
                if "ln" in skip:
                    return
                for hf in range(2):
                    sl = slice(hf * SH, (hf + 1) * SH)
                    # bf16 copy of src for the PE reduction (GpSimd)
                    for kt in range(DT):
                        eng = nc.gpsimd if kt % 2 == 0 else nc.vector
                        eng.tensor_copy(out=hb[:, kt, :],
                                        in_=src[:, kt, sl])
                    st = ps2.tile([33, SH], F32, tag="ln_st")
                    for kt in range(DT):
                        nc.tensor.matmul(st[0:1, :], lhsT=ones_bf,
                                         rhs=hb[:, kt, :],
                                         start=(kt == 0), stop=(kt == DT - 1),
                                         tile_position=(0, 0),
                                         skip_group_check=True)
                    for kt in range(DT):
                        nc.vector.tensor_tensor(out=gT[:, kt, :],
                                                in0=hb[:, kt, :],
                                                in1=hb[:, kt, :],
                                                op=OP.mult)
                    for kt in range(DT):
                        nc.tensor.matmul(st[32:33, :], lhsT=ones_bf,
                                         rhs=gT[:, kt, :],
                                         start=(kt == 0), stop=(kt == DT - 1),
                                         tile_position=(0, 32),
                                         skip_group_check=True)
                    # per-token stats on one partition (f32)
                    mrow = st_pool.tile([1, SH], F32, tag="ln_m")
                    nc.vector.tensor_scalar_mul(out=mrow, in0=st[0:1, :],
                                                scalar1=1.0 / D)
                    vrow = st_pool.tile([1, SH], F32, tag="ln_v")
                    nc.vector.tensor_scalar_mul(out=vrow, in0=st[32:33, :],
                                                scalar1=1.0 / D)
                    msq = st_pool.tile([1, SH], F32, tag="ln_msq")
                    nc.vector.tensor_tensor(out=msq, in0=mrow, in1=mrow,
                                            op=OP.mult)
                    nc.vector.tensor_tensor(out=vrow, in0=vrow, in1=msq,
                                            op=OP.subtract)
                    nc.scalar.activation(out=vrow, in_=vrow, func=AF.Sqrt,
                                         bias=eps_t[0:1], scale=1.0)
                    nc.vector.reciprocal(out=vrow, in_=vrow)      # r
                    nc.vector.tensor_tensor(out=mrow, in0=mrow, in1=vrow,
                                            op=OP.mult)           # m*r
                    nc.gpsimd.partition_broadcast(rb_sb, vrow)
                    nc.gpsimd.partition_broadcast(mb_sb, mrow)
                    # dst = (x*r - m*r) [* g + b]
                    for kt in range(DT):
                        nc.vector.tensor_tensor(out=dst[:, kt, sl],
                                                in0=src[:, kt, sl],
                                                in1=rb_sb, op=OP.mult)
                        nc.vector.tensor_tensor(out=dst[:, kt, sl],
                                                in0=dst[:, kt, sl],
                                                in1=mb_sb, op=OP.subtract)
                        if g_ap is not None:
                            nc.vector.tensor_scalar(
                                out=dst[:, kt, sl], in0=dst[:, kt, sl],
                                scalar1=g_ap[:, kt:kt + 1],
                                scalar2=b_ap[:, kt:kt + 1],
                                op0=OP.mult, op1=OP.add)

            def cast_to_nTb():
                for kt in range(DT):
                    eng = nc.gpsimd if kt % 2 == 0 else nc.vector
                    eng.tensor_copy(out=nTb[:, kt, :],
                                    in_=hT[:, kt, :])

            for rep in range(reps):
                nc.sync.dma_start(
                    out=hT, in_=h0T[:].rearrange("(kt p) t -> p kt t", p=128))
                for l in range(DEPTH):
                    # ---- per-layer small tensors ----
                    if not fast:
                        agg = sm.tile([128, DT], F32, tag="agg")
                        nc.sync.dma_start(out=agg, in_=ag_g[l])
                        agb = sm.tile([128, DT], F32, tag="agb")
                        nc.sync.dma_start(out=agb, in_=ag_b[l])
                        fgg = sm.tile([128, DT], F32, tag="fgg")
                        nc.sync.dma_start(out=fgg, in_=fg_g[l])
                        fgb = sm.tile([128, DT], F32, tag="fgb")
                        nc.sync.dma_start(out=fgb, in_=fg_b[l])
                        bq_t = sm.tile([128, CT], F32, tag="bq")
                        nc.sync.dma_start(out=bq_t, in_=bq[l])
                        bk_t = sm.tile([128, CT], F32, tag="bk")
                        nc.sync.dma_start(out=bk_t, in_=bk[l])
                        bvb = sm.tile([128, CD], F32, tag="bvb")
                        nc.sync.dma_start(out=bvb, in_=_bcast_ap(bv[l], 128))
                        bo_t = sm.tile([128, DT], F32, tag="bo")
                        nc.sync.dma_start(out=bo_t, in_=bo[l])
                        b1_t = sm.tile([128, DT], F32, tag="b1")
                        nc.sync.dma_start(out=b1_t, in_=b1[l])
                        b2_t = sm.tile([128, DT], F32, tag="b2")
                        nc.sync.dma_start(out=b2_t, in_=b2[l])

                    # ---- LN_attn -> nTb (skipped for l>0 in fast mode:
                    # input is already the previous enc-LN output) ----
                    if fast:
                        if l == 0:
                            layer_norm(hT, nTb, None, None)
                        else:
                            cast_to_nTb()
                    else:
                        layer_norm(hT, nTb, agg, agb)

                    # ---- Q, K projections (col-major YT form) ----
                    for (w_dram, b_dram, dstT) in (
                            () if "qkv" in skip else ((wq, bq, qTb),
                                                      (wk, bk, kTb))):
                        w_sb = wp.tile([128, DT, D], BF16, tag="w")
                        nc.sync.dma_start(
                            out=w_sb[:, :, 0:CD],
                            in_=w_dram[l].rearrange("(kt p) c -> p kt c",
                                                    p=128))
                        for wc in range(CT):
                            for hf in range(2):
                                sl = slice(hf * SH, (hf + 1) * SH)
                                acc = ps.tile([128, SH], F32, tag="ps")
                                for kt in range(DT):
                                    nc.tensor.matmul(
                                        acc,
                                        lhsT=w_sb[:, kt,
                                                  wc * 128:(wc + 1) * 128],
                                        rhs=nTb[:, kt, sl],
                                        start=(kt == 0),
                                        stop=(kt == DT - 1))
                                if fast:
                                    nc.vector.tensor_copy(
                                        out=dstT[:, wc, sl], in_=acc)
                                else:
                                    b_t = sm.tile([128, CT], F32,
                                                  tag="bq" if dstT is qTb
                                                  else "bk")
                                    nc.vector.tensor_scalar(
                                        out=dstT[:, wc, sl], in0=acc,
                                        scalar1=b_t[:, wc:wc + 1],
                                        scalar2=None, op0=OP.add)

                    # ---- V projection (token-major) + bias + mask ----
                    w_sb = wp.tile([128, DT, D], BF16, tag="w")
                    nc.sync.dma_start(
                        out=w_sb[:, :, 0:CD],
                        in_=wv[l].rearrange("(kt p) c -> p kt c", p=128))
                    for tt in (() if "qkv" in skip else range(NT)):
                        acc = ps.tile([128, SH], F32, tag="ps")
                        for kt in range(DT):
                            nc.tensor.matmul(
                                acc[:, 0:CD],
                                lhsT=nTb[:, kt, tt * 128:(tt + 1) * 128],
                                rhs=w_sb[:, kt, 0:CD],
                                start=(kt == 0), stop=(kt == DT - 1))
                        if not fast:
                            bvb = sm.tile([128, CD], F32, tag="bvb")
                            nc.vector.tensor_tensor(
                                out=acc[:, 0:CD], in0=acc[:, 0:CD],
                                in1=bvb, op=OP.add)
                        nc.vector.tensor_scalar_mul(
                            out=vtkb[:, tt, :], in0=acc[:, 0:CD],
                            scalar1=m01_sb[:, tt:tt + 1])

                    # ---- attention per head ----
                    for h in (() if "attn" in skip else range(HLOC)):
                        hr = (h % 2) * 64
                        h3 = h // 2
                        for qh in range(2):
                            qsl = slice(qh * SH, (qh + 1) * SH)
                            o_ps = ps.tile([65, SH], F32, tag="ps")
                            for kt2 in range(NT // 2):
                                s_ps = scp.tile([128, 2, SH], F32, tag="sc")
                                for j in range(2):
                                    kt = 2 * kt2 + j
                                    nc.tensor.matmul(
                                        s_ps[:, j, :],
                                        lhsT=kTb[hr:hr + 64, h3,
                                                 kt * 128:(kt + 1) * 128],
                                        rhs=qTb[hr:hr + 64, h3, qsl],
                                        start=True, stop=True)
                                eT = hot.tile([128, 2, SH], BF16, tag="eT")
                                nc.scalar.activation(out=eT, in_=s_ps,
                                                     func=AF.Exp,
                                                     scale=SCALE)
                                for j in range(2):
                                    kt = 2 * kt2 + j
                                    nc.tensor.matmul(
                                        o_ps[0:64, :],
                                        lhsT=vtkb[:, kt,
                                                  h * 64:(h + 1) * 64],
                                        rhs=eT[:, j, :],
                                        start=(kt == 0),
                                        stop=(kt == NT - 1),
                                        tile_position=(0, 0),
                                        skip_group_check=True)
                                    nc.tensor.matmul(
                                        o_ps[64:65, :],
                                        lhsT=m01b[:, kt:kt + 1],
                                        rhs=eT[:, j, :],
                                        start=(kt == 0),
                                        stop=(kt == NT - 1),
                                        tile_position=(0, 64),
                                        skip_group_check=True)
                            rs = st_pool.tile([1, SH], F32, tag="rs")
                            nc.vector.reciprocal(out=rs, in_=o_ps[64:65, :])
                            ou = hot.tile([64, SH], BF16, tag="on")
                            nc.vector.tensor_copy(out=ou, in_=o_ps[0:64, :])
                            for qc in range(SH // 128):
                                # per-token 1/sum as a [128,1] column
                                rT_ps = ps.tile([128, 1], F32, tag="ps")
                                nc.tensor.matmul(
                                    rT_ps,
                                    lhsT=rs[:, qc * 128:(qc + 1) * 128],
                                    rhs=ones_col[0:1, 0:1],
                                    start=True, stop=True)
                                rT_sb = st_pool.tile([128, 1], F32,
                                                     tag="rT")
                                nc.vector.tensor_copy(out=rT_sb, in_=rT_ps)
                                tp_ps = ps.tile([128, 64], BF16, tag="ps")
                                nc.tensor.transpose(
                                    tp_ps, ou[:, qc * 128:(qc + 1) * 128],
                                    ident[0:64, 0:64])
                                nc.vector.tensor_scalar_mul(
                                    out=otok[:, qh * (SH // 128) + qc,
                                             h * 64:(h + 1) * 64],
                                    in0=tp_ps, scalar1=rT_sb)
                        # spill head output to DRAM in flat [h, s, hd] order
                        nc.sync.dma_start(
                            out=o_scr[h].rearrange("(tt p) d -> p tt d",
                                                   p=128),
                            in_=otok[:, :, h * 64:(h + 1) * 64])

                    # ---- scrambled [SH, D] view, transpose to col-major ----
                    oview = o_scr[:].rearrange("h s d -> (h s d)").rearrange(
                        "(r c) -> r c", c=D)
                    for rt in (() if "obounce" in skip else range(SH // 128)):
                        ov_tok = hot.tile([128, D], BF16, tag="ov_tok")
                        nc.sync.dma_start(
                            out=ov_tok, in_=oview[rt * 128:(rt + 1) * 128, :])
                        for ct in range(DT):
                            tp_ps = ps.tile([128, 128], BF16, tag="ps")
                            nc.tensor.transpose(
                                tp_ps, ov_tok[:, ct * 128:(ct + 1) * 128],
                                ident)
                            nc.vector.tensor_copy(
                                out=ovT[:, ct, rt * 128:(rt + 1) * 128],
                                in_=tp_ps)

                    # ---- Wo GEMM -> delta for my 512 rows (+bo) -> dTb ----
                    w_sb = wp.tile([128, DT, D], BF16, tag="w")
                    nc.sync.dma_start(
                        out=w_sb,
                        in_=wo[l].rearrange("(ct p) w -> p ct w", p=128))
                    for wc in (() if "wo" in skip else range(DT)):
                        acc = ps.tile([128, SH], F32, tag="ps")
                        for ct in range(DT):
                            nc.tensor.matmul(
                                acc,
                                lhsT=w_sb[:, ct, wc * 128:(wc + 1) * 128],
                                rhs=ovT[:, ct, :],
                                start=(ct == 0), stop=(ct == DT - 1))
                        if fast:
                            nc.vector.tensor_copy(out=dTb[:, wc, :], in_=acc)
                        else:
                            bo_t = sm.tile([128, DT], F32, tag="bo")
                            nc.vector.tensor_scalar(
                                out=dTb[:, wc, :], in0=acc,
                                scalar1=bo_t[:, wc:wc + 1], scalar2=None,
                                op0=OP.add)

                    # ---- AllGather deltas within the pair (bf16) ----
                    if "ag" not in skip:
                        ci = cc_in[rep * DEPTH + l]
                        co = cc_out[rep * DEPTH + l]
                        nc.sync.dma_start(
                            out=ci[:].rearrange("(kt p) t -> p kt t", p=128),
                            in_=dTb)
                        if use_cc:
                            nc.gpsimd.collective_compute(
                                "AllGather", OP.bypass, ins=[ci[:]],
                                outs=[co[:]], replica_groups=groups)
                        else:
                            # timeline-sim variant: fake exchange locally
                            nc.sync.dma_start(out=co[0:D, :], in_=ci[:])
                            nc.sync.dma_start(out=co[D:2 * D, :], in_=ci[:])
                        cov = co[:].rearrange("(blk kt p) t -> blk p kt t",
                                              blk=2, p=128)
                        dTf = sb_dT_bf
                        for blk in range(2):
                            nc.sync.dma_start(
                                out=dTf[:, :, blk * SH:(blk + 1) * SH],
                                in_=cov[blk])
                        # full residual: h += delta (bf16 in1)
                        for kt in range(DT):
                            nc.vector.tensor_tensor(out=hT[:, kt, :],
                                                    in0=hT[:, kt, :],
                                                    in1=dTf[:, kt, :],
                                                    op=OP.add)

                    # ---- post-attn enc LN (full, in place) ----
                    layer_norm(hT, hT, None if fast else eng_sb,
                               None if fast else enb_sb)

                    # ---- FFN (full tokens, redundant across the pair) ----
                    if fast:
                        cast_to_nTb()   # ffn-LN == identity here
                    else:
                        layer_norm(hT, nTb, fgg, fgb)
                    w1_sb = wp.tile([128, DT, D], BF16, tag="w")
                    nc.sync.dma_start(
                        out=w1_sb,
                        in_=w1[l].rearrange("(kt p) c -> p kt c", p=128))
                    w2_sb = wp.tile([128, DT, D], BF16, tag="w")
                    nc.sync.dma_start(
                        out=w2_sb,
                        in_=w2[l].rearrange("(kt p) c -> p kt c", p=128))
                    for hf in (() if "ffn" in skip else range(2)):
                        sl = slice(hf * SH, (hf + 1) * SH)
                        for hc in range(DT):
                            acc = ps.tile([128, SH], F32, tag="ps")
                            for kt in range(DT):
                                nc.tensor.matmul(
                                    acc,
                                    lhsT=w1_sb[:, kt,
                                               hc * 128:(hc + 1) * 128],
                                    rhs=nTb[:, kt, sl],
                                    start=(kt == 0), stop=(kt == DT - 1))
                            if fast:
                                nc.scalar.activation(out=gT[:, hc, :],
                                                     in_=acc, func=AF.Gelu,
                                                     scale=1.0)
                            else:
                                b1_t = sm.tile([128, DT], F32, tag="b1")
                                nc.scalar.activation(out=gT[:, hc, :],
                                                     in_=acc, func=AF.Gelu,
                                                     bias=b1_t[:, hc:hc + 1],
                                                     scale=1.0)
                        for wc in range(DT):
                            acc = ps.tile([128, SH], F32, tag="ps")
                            for kt in range(DT):
                                nc.tensor.matmul(
                                    acc,
                                    lhsT=w2_sb[:, kt,
                                               wc * 128:(wc + 1) * 128],
                                    rhs=gT[:, kt, :],
                                    start=(kt == 0), stop=(kt == DT - 1))
                            if not fast:
                                b2_t = sm.tile([128, DT], F32, tag="b2")
                                nc.vector.tensor_scalar(
                                    out=acc, in0=acc,
                                    scalar1=b2_t[:, wc:wc + 1],
                                    scalar2=None, op0=OP.add)
                            nc.vector.tensor_tensor(
                                out=hT[:, wc, sl], in0=hT[:, wc, sl],
                                in1=acc, op=OP.add)

                    # ---- post-FFN enc LN (full, in place) ----
                    layer_norm(hT, hT, None if fast else eng_sb,
                               None if fast else enb_sb)

            nc.sync.dma_start(
                out=outT[:].rearrange("(kt p) t -> p kt t", p=128),
                in_=hT)

    nc.finalize()
    return nc


def _per_core_inputs(inputs, core):
    """Host-side sharding: build this core's input map."""
    b, tp = core // 2, core % 2
    csl = slice(tp * CD, (tp + 1) * CD)

    x = np.asarray(inputs["x"])
    emb = np.asarray(inputs["emb"], dtype=np.float32)
    pos = np.asarray(inputs["pos_embed"], dtype=np.float32)[0]
    h0 = emb[x[b]] + pos                                # [S, D]
    mask = np.asarray(inputs["mask"])[b, 0, 0]          # [S]

    def pp(v, dt_):  # [DEPTH, 768] -> [DEPTH, 128, 6]
        v = np.asarray(v, dtype=np.float32)
        return np.ascontiguousarray(
            v.reshape(DEPTH, dt_, 128).transpose(0, 2, 1))

    def bf(v):
        return np.ascontiguousarray(
            np.asarray(v, np.float32).astype(ml_dtypes.bfloat16))

    return {
        "h0T": np.ascontiguousarray(h0.T),
        "m01": np.ascontiguousarray(
            mask.reshape(NT, 128).T.astype(np.float32)),
        "wq": bf(np.asarray(inputs["Wq"], np.float32)[:, :, csl]),
        "wk": bf(np.asarray(inputs["Wk"], np.float32)[:, :, csl]),
        "wv": bf(np.asarray(inputs["Wv"], np.float32)[:, :, csl]),
        "wo": bf(inputs["Wo"]),
        "w1": bf(inputs["W1"]),
        "w2": bf(inputs["W2"]),
        "bq": np.ascontiguousarray(
            np.asarray(inputs["bq"], np.float32)[:, csl]
            .reshape(DEPTH, CT, 128).transpose(0, 2, 1)),
        "bk": np.ascontiguousarray(
            np.asarray(inputs["bk"], np.float32)[:, csl]
            .reshape(DEPTH, CT, 128).transpose(0, 2, 1)),
        "bv": np.ascontiguousarray(
            np.asarray(inputs["bv"], np.float32)[:, csl]
            .reshape(DEPTH, 1, CD)),
        "bo": pp(inputs["bo"], DT),
        "b1": pp(inputs["b1"], DT),
        "b2": pp(inputs["b2"], DT),
        "ag_g": pp(inputs["attn_g"], DT),
        "ag_b": pp(inputs["attn_b"], DT),
        "fg_g": pp(inputs["ff_g"], DT),
        "fg_b": pp(inputs["ff_b"], DT),
        "en_g": np.ascontiguousarray(
            np.asarray(inputs["enc_g"], np.float32).reshape(DT, 128).T),
        "en_b": np.ascontiguousarray(
            np.asarray(inputs["enc_b"], np.float32).reshape(DT, 128).T),
    }


def _is_fast_affine(inputs):
    z = lambda k: not np.any(np.asarray(inputs[k], np.float32))
    o = lambda k: np.all(np.asarray(inputs[k], np.float32) == 1.0)
    return (z("bq") and z("bk") and z("bv") and z("bo") and z("b1")
            and z("b2") and z("attn_b") and z("ff_b") and z("enc_b")
            and o("attn_g") and o("ff_g") and o("enc_g"))


_CACHE = {}


def _get_nc(reps=1, fast=True):
    key = ("nc", reps, fast)
    if key not in _CACHE:
        _CACHE[key] = build(reps=reps, fast=fast)
    return _CACHE[key]


def kernel(**inputs):
    fast = _is_fast_affine(inputs)
    nc = _get_nc(reps=1, fast=fast)
    in_maps = [_per_core_inputs(inputs, c) for c in range(8)]
    res = run_bass_kernel_spmd(nc, in_maps, list(range(8)))
    out = np.empty((B, S, D), dtype=np.float32)
    for b in range(B):
        out[b] = res.results[2 * b]["outT"].T
    return out
